# revision 27
# baseline (speedup 1.0000x reference)
"""CVLoss Trainium2 kernel.

Computes the per-neuron coefficient-of-variation (CV) of inter-spike
intervals over a (B*T, N) spike train and the MSE loss against target CVs.

Sharding: neuron/model parallel — 8 cores x 128 neurons, each core gets its
contiguous (32768, 128) slice of the time-flattened train. Inside a core the
time axis is split into two independent halves ("chains" A/B) whose scans
interleave on the vector engine; the host stitches them at the boundary.

Per-core device pipeline (each chain processed in chunks):
  - DMA loads a chunk time-major ([128 time, blocks, 128 neurons]).
  - GPSIMD computes notm = 1 - m, downcast to fp16 (spikes are 0/1, exact).
  - PE (a) transposes each 128x128 block to neuron-major PSUM via regular
    matmul (notm^T = notm.T @ I, fp32 PSUM), and (b) computes batched
    "nibble" matmuls: for every 4-timestep window, sum(notm * 2^(t%4)) — an
    exact, invertible 4-bit window mask. The host uses the nibbles ONLY for
    spike counts (popcount) and first-spike indices (first set bit).
  - DVE runs the age recurrence a_t = (a_{t-1}+1)*(1-m_t) with the hardware
    tensor_tensor_scan (reads notm^T straight from PSUM; carries chain
    across chunks via its last element). This is the bottleneck: the scan
    executes at 2 cycles/element regardless of dtype or ALU ops (measured),
    so everything else is sized to hide beneath it.
  - ACT accumulates sum(ages) per chunk (activation accum_out).

The ISI statistics collapse to these sums via a telescoping identity
(with b = 1-m, note ((a+1)b)^2 = a'^2, so the masked square-gap sum
telescopes):
    sum over spikes of gap^2 = 1 - (a_end+1)^2 + 2*sum(ages) + T
including one blind first-spike gap (t_first+1)^2 that the host removes.
Per neuron the device therefore only returns sum(ages) per chunk, a_end per
chain, and the nibble masks; the host (float32, replicating the reference
op-for-op) computes mean ISI = (t_last-t_first)/(k-1), the unbiased
variance, CV, and the masked MSE against target_cv.

fp16 ages are exact for gaps <= 2048 steps; at the 2% spike rate of this
workload the maximum observed age is ~700 (P[gap>2048] ~ 1e-18 per site).
Neurons with k < 3 spikes are excluded by the reference's valid mask, which
the host replicates, so pathological all-quiet neurons cannot corrupt the
loss.
"""

import numpy as np

import concourse.bacc as bacc
import concourse.bass as bass
import concourse.mybir as mybir
import concourse.tile as tile
from concourse import bass_utils

B, T_STEP, N = 16, 2048, 1024
TT = B * T_STEP              # 32768 total timesteps per neuron
NCORES = 8
NLOC = N // NCORES           # 128 neurons per core
CHUNK = 2048                 # main chunk size
# two independent time-half chains per core (host-stitched); head chunks
# small so each chain's scans start early
CHAIN_SIZES = [512] * 4 + [2048] * 7
assert sum(CHAIN_SIZES) == TT // 2
NCHUNK = TT // CHUNK         # 16 (nibble-layout unit)
NBLK = CHUNK // 128          # 16 blocks per full chunk


def _chain_schedule(tt=TT):
    if tt == TT:
        return CHAIN_SIZES
    return [CHUNK] * (tt // 2 // CHUNK)

F32 = mybir.dt.float32
F16 = mybir.dt.float16
AF = mybir.ActivationFunctionType
ALU = mybir.AluOpType
AX = mybir.AxisListType

# stats layout (columns of the [128, NSTAT] f32 output):
#   [0:23)     sum(ages) per chunk, chain A then chain B (zero-padded)
#   [46]       a_end of chain A (age at t = TT/2 - 1, chain-local)
#   [47]       a_end of chain B (age at t = TT - 1, chain-local)
SA0 = 0
SA_B = 23
AEND0 = 46
NSTAT = 48

# bitmask output: [128, NCHUNK*1024] f16; per chunk a [128, 1024] block laid
# out as partition p = 64*(blk%2) + c (c < 32 real, else zero), free =
# (blk//2)*128 + n, holding sum_{j<4} notm[t,n]*2^j for t = 128*blk+4*c+j.
BM_W = 1024


def _wmask_np():
    """[128, 64] fp16 nibble weights: W[t, c] = (t//4 == c) * 2^(t%4),
    columns 32..63 zero-padding (PE col-group alignment)."""
    w = np.zeros((128, 64), dtype=np.float16)
    for t in range(128):
        w[t, t // 4] = np.float16(2.0 ** (t % 4))
    return w


def build_kernel(tt=TT):
    nchunk = tt // CHUNK
    nc = bacc.Bacc("TRN2", target_bir_lowering=False, debug=False)
    spikes = nc.dram_tensor("spikes", [tt, NLOC], F32, kind="ExternalInput")
    ident = nc.dram_tensor("ident", [128, 128], F16, kind="ExternalInput")
    wmask = nc.dram_tensor("wmask", [128, 64], F16, kind="ExternalInput")
    stats = nc.dram_tensor("stats", [128, NSTAT], F32, kind="ExternalOutput")
    bmask = nc.dram_tensor("bmask", [128, NCHUNK * BM_W], F16, kind="ExternalOutput")

    sp = spikes.ap()

    with tile.TileContext(nc) as tc:
        with (
            tc.tile_pool(name="static", bufs=1) as static_pool,
            tc.tile_pool(name="raw", bufs=5) as raw_pool,
            tc.tile_pool(name="notm", bufs=4) as notm_pool,
            tc.tile_pool(name="ages", bufs=3) as ages_pool,
            tc.tile_pool(name="junk", bufs=1) as junk_pool,
            tc.tile_pool(name="bmsb", bufs=2) as bm_pool,
            tc.tile_pool(name="stats", bufs=1) as stats_pool,
            tc.tile_pool(name="psum", bufs=2, space="PSUM") as psum_pool,
            tc.tile_pool(name="psbm", bufs=2, space="PSUM") as psbm_pool,
            # PSUM budget: mt [128,1024]f32 = 2 banks x2 bufs + bm
            # [128,1024]f32 = 2 banks x2 bufs = 8 banks total
        ):
            chain_sizes = _chain_schedule(tt)
            half_tt = tt // 2
            # issue the first chunk DMA of each chain before static loads so
            # the pipeline ramp is not gated on them
            raw_first = {}
            for ci, base in enumerate((0, half_tt)):
                r0 = raw_pool.tile([128, NBLK, 128], F32, tag="raw")
                nb0 = chain_sizes[0] // 128
                nc.sync.dma_start(
                    r0[:, :nb0, :],
                    sp[base:base + chain_sizes[0], :].rearrange(
                        "(a p) n -> p a n", p=128
                    ),
                )
                raw_first[ci] = r0

            ident_sb = static_pool.tile([128, 128], F16)
            nc.sync.dma_start(ident_sb[:], ident.ap())
            wmask_sb = static_pool.tile([128, 64], F16)
            nc.sync.dma_start(wmask_sb[:], wmask.ap())
            ones_sb = static_pool.tile([128, CHUNK // 2], F16)
            nc.gpsimd.memset(ones_sb[:], 1.0)

            statsb = stats_pool.tile([128, NSTAT], F32)
            nc.gpsimd.memset(statsb[:], 0.0)
            junk = junk_pool.tile([128, CHUNK // 2], F16)

            prev_ages = [None, None]
            prev_half = [CHUNK // 2, CHUNK // 2]
            t0s = [0, half_tt]
            n_sa = [SA0, SA_B]
            for step, csize in enumerate(chain_sizes):
                nblk = csize // 128
                for ci in range(2):
                    t0 = t0s[ci]
                    hb = nblk // 2
                    if step == 0:
                        raw = raw_first[ci]
                    else:
                        raw = raw_pool.tile([128, NBLK, 128], F32, tag="raw")
                        for hh in range(2):
                            nc.sync.dma_start(
                                raw[:, hh * hb:(hh + 1) * hb, :],
                                sp[t0 + hh * csize // 2:
                                   t0 + (hh + 1) * csize // 2, :].rearrange(
                                    "(a p) n -> p a n", p=128
                                ),
                            )
                    # notm = 1 - m, fp16 (GPSIMD); split per half so PE can
                    # start on h0 before the whole chunk has been cast
                    notm = notm_pool.tile([128, NBLK, 128], F16, tag="notm")
                    for hh in range(2):
                        nc.gpsimd.tensor_scalar(
                            notm[:, hh * hb:(hh + 1) * hb, :],
                            raw[:, hh * hb:(hh + 1) * hb, :], -1.0, 1.0,
                            ALU.mult, ALU.add
                        )

                    # PE nibble matmuls (batched, <=1 psum bank each)
                    bm = psbm_pool.tile([128, BM_W], F32, tag="bm")
                    notm_qr = notm[:, :nblk, :].rearrange(
                        "p (q r) n -> p r q n", r=2
                    )
                    qtot = nblk // 2
                    for r in range(2):
                        for qh in range(0, qtot, 4):
                            qn = min(4, qtot - qh)
                            nc.tensor.matmul(
                                bm[64 * r:64 * (r + 1),
                                   qh * 128:(qh + qn) * 128],
                                wmask_sb[:],
                                notm_qr[:, r, qh:qh + qn],
                            )
                    # bitmask evac PSUM -> SBUF (fp16 exact: values <= 15)
                    bw = qtot * 128
                    bm_off = t0 // 2
                    bmsb = bm_pool.tile([128, BM_W], F16, tag="bmsb")
                    nc.scalar.copy(bmsb[:, :bw], bm[:, :bw])
                    nc.sync.dma_start(
                        bmask.ap()[:, bm_off:bm_off + bw], bmsb[:, :bw]
                    )

                    # transpose (regular matmul) + age scan per half chunk
                    half = csize // 2
                    for h in range(2):
                        mt = psum_pool.tile([128, CHUNK // 2], F32, tag="mt")
                        for b2 in range(half // 128):
                            blk = h * (half // 128) + b2
                            nc.tensor.matmul(
                                mt[:, b2 * 128:(b2 + 1) * 128],
                                notm[:, blk, :],
                                ident_sb[:],
                            )
                        ages = ages_pool.tile(
                            [128, CHUNK // 2], F16, tag="ages"
                        )
                        pa = prev_ages[ci]
                        a_init = (
                            0.0 if pa is None
                            else pa[:, prev_half[ci] - 1:prev_half[ci]]
                        )
                        nc.vector.tensor_tensor_scan(
                            ages[:, :half], ones_sb[:, :half], mt[:, :half],
                            a_init, op0=ALU.add, op1=ALU.mult,
                        )
                        # sum(ages) via ACT accumulate (junk elementwise out)
                        sa = n_sa[ci]
                        nc.scalar.activation(
                            junk[:, :half], ages[:, :half], AF.Identity,
                            bias=0.0, scale=1.0,
                            accum_out=statsb[:, sa:sa + 1],
                        )
                        n_sa[ci] += 1
                        prev_ages[ci] = ages
                        prev_half[ci] = half
                    t0s[ci] += csize
            # per-chain a_end (chain-local age at the chain's last step)
            for ci in range(2):
                nc.vector.tensor_copy(
                    statsb[:, AEND0 + ci:AEND0 + ci + 1],
                    prev_ages[ci][:, prev_half[ci] - 1:prev_half[ci]],
                )
            nc.sync.dma_start(stats.ap(), statsb[:])

    nc.compile()
    return nc


_CACHE = {}


def _get_nc():
    if "nc" not in _CACHE:
        _CACHE["nc"] = build_kernel()
    return _CACHE["nc"]


_POP = np.array([bin(i).count("1") for i in range(16)], dtype=np.int64)


def _decode_bitmasks(bm, tt=TT):
    """bm: [128, >=tt/2] f16 of notm-nibbles -> per-time-half (k, t_f).

    Per chunk of size csize at t0, a [128, csize/2] block at free offset
    t0/2: partition p = 64*r + c (c < 32 real window), free = q*128 + n,
    blk = 2*q + r, covering t = t0 + 128*blk + 4*c + j with value
    sum(notm * 2^j); spike nibble is 15 - value.
    """
    parts = []
    off = 0
    bmv = np.asarray(bm, dtype=np.float64)
    sched = _chain_schedule(tt) * 2      # time order: chain A then chain B
    for csize in sched:
        bw = csize // 2
        qtot = csize // 256
        v = np.round(bmv[:, off:off + bw]).astype(np.int64)
        v = v.reshape(2, 64, qtot, 128)          # [r, c, q, n]
        m_nib = (15 - v[:, :32]).transpose(3, 2, 0, 1)  # [n, q, r, c]
        parts.append(m_nib.reshape(128, qtot * 2 * 32))
        off += bw
    flat = np.concatenate(parts, axis=1)         # [n, tt/4] time-ordered

    def half_stats(nib, base):
        k = _POP[nib].sum(axis=1)
        any_nib = nib > 0
        first_nib = np.argmax(any_nib, axis=1)
        has = any_nib.any(axis=1)
        nib_val = nib[np.arange(128), first_nib]
        low = np.zeros(128, dtype=np.int64)
        for j in range(3, -1, -1):
            low = np.where((nib_val >> j) & 1 == 1, j, low)
        t_f = np.where(has, base + first_nib * 4 + low, tt)
        return k.astype(np.float64), t_f.astype(np.float64)

    hn = flat.shape[1] // 2
    kA, tfA = half_stats(flat[:, :hn], 0)
    kB, tfB = half_stats(flat[:, hn:], tt // 2)
    return kA, tfA, kB, tfB


def _finalize(stats_list, bmask_list, target_cv, tt=TT):
    """Stitch the two time-half chains and compute the loss (host, f32)."""
    f32 = np.float32
    half = tt / 2.0
    k_l, tf_l, tl_l, s2_l = [], [], [], []
    nchain = len(_chain_schedule(tt))
    for st, bm in zip(stats_list, bmask_list):
        st = np.asarray(st, dtype=np.float64)
        sum_aA = st[:, SA0:SA0 + 2 * nchain].sum(axis=1)
        sum_aB = st[:, SA_B:SA_B + 2 * nchain].sum(axis=1)
        a_endA = st[:, AEND0]
        a_endB = st[:, AEND0 + 1]
        kA, tfA, kB, tfB = _decode_bitmasks(bm, tt)
        hasA, hasB = kA > 0, kB > 0
        # per-chain blind sums (chain-local time, init age 0)
        s2A = 1.0 - (a_endA + 1.0) ** 2 + 2.0 * sum_aA + half
        s2B = 1.0 - (a_endB + 1.0) ** 2 + 2.0 * sum_aB + half
        # remove each chain's blind first gap ((local t_f)+1)^2
        s2A = np.where(hasA, s2A - (tfA + 1.0) ** 2, 0.0)
        s2B = np.where(hasB, s2B - (tfB - half + 1.0) ** 2, 0.0)
        tlA = half - 1.0 - a_endA        # global (== local here)
        tlB = tt - 1.0 - a_endB
        # boundary gap between the halves
        s2 = s2A + s2B + np.where(
            hasA & hasB, (tfB - tlA) ** 2, 0.0
        )
        k = kA + kB
        t_f = np.where(hasA, tfA, tfB)   # == tt when no spikes at all
        t_l = np.where(hasB, tlB, tlA)   # == -1 when no spikes at all
        k_l.append(k); tf_l.append(t_f); tl_l.append(t_l); s2_l.append(s2)
    k = np.concatenate(k_l).astype(f32)
    t_f = np.concatenate(tf_l)
    t_l = np.concatenate(tl_l)
    s2 = np.concatenate(s2_l).astype(f32)
    tgt = np.asarray(target_cv, dtype=f32)

    n_isi = k - f32(1.0)
    sum_g = (t_l - t_f).astype(f32)
    mean = sum_g / np.maximum(n_isi, f32(1.0))
    var = (s2 - n_isi * mean * mean) / np.maximum(n_isi - f32(1.0), f32(1.0))
    std = np.sqrt(np.maximum(var, f32(0.0)).astype(f32))
    valid = (k >= f32(3.0)) & (mean > f32(0.0))
    cv = np.where(valid, std / np.where(mean > f32(0.0), mean, f32(1.0)), f32(0.0))
    sq = np.where(valid, (cv - tgt) ** 2, f32(0.0)).astype(f32)
    nvalid = valid.astype(f32).sum(dtype=f32)
    loss = np.where(
        nvalid > f32(0.0), sq.sum(dtype=f32) / np.maximum(nvalid, f32(1.0)), f32(0.0)
    )
    return np.asarray(loss, dtype=np.float32)


_IDENT = np.eye(128, dtype=np.float16)
_WMASK = _wmask_np()


def make_in_maps(output_spikes):
    s = np.asarray(output_spikes, dtype=np.float32).reshape(TT, N)
    return [
        {
            "spikes": np.ascontiguousarray(s[:, d * NLOC:(d + 1) * NLOC]),
            "ident": _IDENT,
            "wmask": _WMASK,
        }
        for d in range(NCORES)
    ]


def kernel(output_spikes, target_cv, _trace=False):
    nc = _get_nc()
    in_maps = make_in_maps(output_spikes)
    res = bass_utils.run_bass_kernel_spmd(
        nc, in_maps, core_ids=list(range(NCORES)), trace=_trace
    )
    _CACHE["last_result"] = res
    stats_list = [res.results[d]["stats"] for d in range(NCORES)]
    bmask_list = [res.results[d]["bmask"] for d in range(NCORES)]
    return _finalize(stats_list, bmask_list, target_cv)


# revision 28
# speedup vs baseline: 1.0391x; 1.0391x over previous
"""CVLoss Trainium2 kernel.

Computes the per-neuron coefficient-of-variation (CV) of inter-spike
intervals over a (B*T, N) spike train and the MSE loss against target CVs.

Sharding: neuron/model parallel — 8 cores x 128 neurons, each core gets its
contiguous (32768, 128) slice of the time-flattened train. Inside a core the
time axis is split into two independent halves ("chains" A/B) whose scans
interleave on the vector engine; the host stitches them at the boundary.

Per-core device pipeline (each chain processed in chunks):
  - DMA loads a chunk time-major ([128 time, blocks, 128 neurons]).
  - GPSIMD computes notm = 1 - m, downcast to fp16 (spikes are 0/1, exact).
  - PE (a) transposes each 128x128 block to neuron-major PSUM via regular
    matmul (notm^T = notm.T @ I, fp32 PSUM), and (b) computes batched
    "nibble" matmuls: for every 4-timestep window, sum(notm * 2^(t%4)) — an
    exact, invertible 4-bit window mask. The host uses the nibbles ONLY for
    spike counts (popcount) and first-spike indices (first set bit).
  - DVE runs the age recurrence a_t = (a_{t-1}+1)*(1-m_t) with the hardware
    tensor_tensor_scan (reads notm^T straight from PSUM; carries chain
    across chunks via its last element). This is the bottleneck: the scan
    executes at 2 cycles/element regardless of dtype or ALU ops (measured),
    so everything else is sized to hide beneath it.
  - ACT accumulates sum(ages) per chunk (activation accum_out).

The ISI statistics collapse to these sums via a telescoping identity
(with b = 1-m, note ((a+1)b)^2 = a'^2, so the masked square-gap sum
telescopes):
    sum over spikes of gap^2 = 1 - (a_end+1)^2 + 2*sum(ages) + T
including one blind first-spike gap (t_first+1)^2 that the host removes.
Per neuron the device therefore only returns sum(ages) per chunk, a_end per
chain, and the nibble masks; the host (float32, replicating the reference
op-for-op) computes mean ISI = (t_last-t_first)/(k-1), the unbiased
variance, CV, and the masked MSE against target_cv.

fp16 ages are exact for gaps <= 2048 steps; at the 2% spike rate of this
workload the maximum observed age is ~700 (P[gap>2048] ~ 1e-18 per site).
Neurons with k < 3 spikes are excluded by the reference's valid mask, which
the host replicates, so pathological all-quiet neurons cannot corrupt the
loss.
"""

import numpy as np

import concourse.bacc as bacc
import concourse.bass as bass
import concourse.mybir as mybir
import concourse.tile as tile
from concourse import bass_utils

B, T_STEP, N = 16, 2048, 1024
TT = B * T_STEP              # 32768 total timesteps per neuron
NCORES = 8
NLOC = N // NCORES           # 128 neurons per core
CHUNK = 2048                 # main chunk size
# two independent time-half chains per core (host-stitched); head chunks
# small so each chain's scans start early
CHAIN_SIZES = [512] * 4 + [2048] * 7
assert sum(CHAIN_SIZES) == TT // 2
NCHUNK = TT // CHUNK         # 16 (nibble-layout unit)
NBLK = CHUNK // 128          # 16 blocks per full chunk


def _chain_schedule(tt=TT):
    if tt == TT:
        return CHAIN_SIZES
    return [CHUNK] * (tt // 2 // CHUNK)

F32 = mybir.dt.float32
F16 = mybir.dt.float16
AF = mybir.ActivationFunctionType
ALU = mybir.AluOpType
AX = mybir.AxisListType

# stats layout (columns of the [128, NSTAT] f32 output):
#   [0:23)     sum(ages) per chunk, chain A then chain B (zero-padded)
#   [46]       a_end of chain A (age at t = TT/2 - 1, chain-local)
#   [47]       a_end of chain B (age at t = TT - 1, chain-local)
SA0 = 0
SA_B = 23
AEND0 = 46
NSTAT = 48

# bitmask output: [128, NCHUNK*1024] f16; per chunk a [128, 1024] block laid
# out as partition p = 64*(blk%2) + c (c < 32 real, else zero), free =
# (blk//2)*128 + n, holding sum_{j<4} notm[t,n]*2^j for t = 128*blk+4*c+j.
BM_W = 1024


def _wmask_np():
    """[128, 64] fp16 nibble weights: W[t, c] = (t//4 == c) * 2^(t%4),
    columns 32..63 zero-padding (PE col-group alignment)."""
    w = np.zeros((128, 64), dtype=np.float16)
    for t in range(128):
        w[t, t // 4] = np.float16(2.0 ** (t % 4))
    return w


def build_kernel(tt=TT):
    nchunk = tt // CHUNK
    nc = bacc.Bacc("TRN2", target_bir_lowering=False, debug=False)
    spikes = nc.dram_tensor("spikes", [tt, NLOC], F32, kind="ExternalInput")
    ident = nc.dram_tensor("ident", [128, 128], F16, kind="ExternalInput")
    wmask = nc.dram_tensor("wmask", [128, 64], F16, kind="ExternalInput")
    stats = nc.dram_tensor("stats", [128, NSTAT], F32, kind="ExternalOutput")
    bmask = nc.dram_tensor("bmask", [128, NCHUNK * BM_W], F16, kind="ExternalOutput")

    sp = spikes.ap()

    with tile.TileContext(nc) as tc:
        with (
            tc.tile_pool(name="static", bufs=1) as static_pool,
            tc.tile_pool(name="raw", bufs=5) as raw_pool,
            tc.tile_pool(name="notm", bufs=4) as notm_pool,
            tc.tile_pool(name="ages", bufs=3) as ages_pool,
            tc.tile_pool(name="junk", bufs=1) as junk_pool,
            tc.tile_pool(name="bmsb", bufs=2) as bm_pool,
            tc.tile_pool(name="stats", bufs=1) as stats_pool,
            tc.tile_pool(name="psum", bufs=2, space="PSUM") as psum_pool,
            tc.tile_pool(name="psbm", bufs=2, space="PSUM") as psbm_pool,
            # PSUM budget: mt [128,1024]f32 = 2 banks x2 bufs + bm
            # [128,1024]f32 = 2 banks x2 bufs = 8 banks total
        ):
            chain_sizes = _chain_schedule(tt)
            half_tt = tt // 2
            # issue the first chunk DMA of each chain before static loads so
            # the pipeline ramp is not gated on them
            raw_first = {}
            for ci, base in enumerate((0, half_tt)):
                r0 = raw_pool.tile([128, NBLK, 128], F32, tag="raw")
                nb0 = chain_sizes[0] // 128
                nc.sync.dma_start(
                    r0[:, :nb0, :],
                    sp[base:base + chain_sizes[0], :].rearrange(
                        "(a p) n -> p a n", p=128
                    ),
                )
                raw_first[ci] = r0

            ident_sb = static_pool.tile([128, 128], F16)
            nc.sync.dma_start(ident_sb[:], ident.ap())
            wmask_sb = static_pool.tile([128, 64], F16)
            nc.sync.dma_start(wmask_sb[:], wmask.ap())
            ones_sb = static_pool.tile([128, CHUNK // 2], F16)
            nc.gpsimd.memset(ones_sb[:], 1.0)

            statsb = stats_pool.tile([128, NSTAT], F32)
            nc.gpsimd.memset(statsb[:], 0.0)
            junk = junk_pool.tile([128, CHUNK // 2], F16)

            prev_ages = [None, None]
            prev_half = [CHUNK // 2, CHUNK // 2]
            t0s = [0, half_tt]
            n_sa = [SA0, SA_B]
            for step, csize in enumerate(chain_sizes):
                nblk = csize // 128
                for ci in range(2):
                    t0 = t0s[ci]
                    if step == 0:
                        raw = raw_first[ci]
                    else:
                        raw = raw_pool.tile([128, NBLK, 128], F32, tag="raw")
                        nc.sync.dma_start(
                            raw[:, :nblk, :],
                            sp[t0:t0 + csize, :].rearrange(
                                "(a p) n -> p a n", p=128
                            ),
                        )
                    # notm = 1 - m, fp16 (GPSIMD, 1-input ~line-rate)
                    notm = notm_pool.tile([128, NBLK, 128], F16, tag="notm")
                    nc.gpsimd.tensor_scalar(
                        notm[:, :nblk, :], raw[:, :nblk, :], -1.0, 1.0,
                        ALU.mult, ALU.add
                    )

                    # PE nibble matmuls (batched, <=1 psum bank each)
                    bm = psbm_pool.tile([128, BM_W], F32, tag="bm")
                    notm_qr = notm[:, :nblk, :].rearrange(
                        "p (q r) n -> p r q n", r=2
                    )
                    qtot = nblk // 2
                    for r in range(2):
                        for qh in range(0, qtot, 4):
                            qn = min(4, qtot - qh)
                            nc.tensor.matmul(
                                bm[64 * r:64 * (r + 1),
                                   qh * 128:(qh + qn) * 128],
                                wmask_sb[:],
                                notm_qr[:, r, qh:qh + qn],
                            )
                    # bitmask evac PSUM -> SBUF (fp16 exact: values <= 15)
                    bw = qtot * 128
                    bm_off = t0 // 2
                    bmsb = bm_pool.tile([128, BM_W], F16, tag="bmsb")
                    nc.scalar.copy(bmsb[:, :bw], bm[:, :bw])
                    nc.sync.dma_start(
                        bmask.ap()[:, bm_off:bm_off + bw], bmsb[:, :bw]
                    )

                    # transpose (regular matmul) + age scan per half chunk
                    half = csize // 2
                    for h in range(2):
                        mt = psum_pool.tile([128, CHUNK // 2], F32, tag="mt")
                        for b2 in range(half // 128):
                            blk = h * (half // 128) + b2
                            nc.tensor.matmul(
                                mt[:, b2 * 128:(b2 + 1) * 128],
                                notm[:, blk, :],
                                ident_sb[:],
                            )
                        ages = ages_pool.tile(
                            [128, CHUNK // 2], F16, tag="ages"
                        )
                        pa = prev_ages[ci]
                        a_init = (
                            0.0 if pa is None
                            else pa[:, prev_half[ci] - 1:prev_half[ci]]
                        )
                        nc.vector.tensor_tensor_scan(
                            ages[:, :half], ones_sb[:, :half], mt[:, :half],
                            a_init, op0=ALU.add, op1=ALU.mult,
                        )
                        # sum(ages) via ACT accumulate (junk elementwise out)
                        sa = n_sa[ci]
                        nc.scalar.activation(
                            junk[:, :half], ages[:, :half], AF.Identity,
                            bias=0.0, scale=1.0,
                            accum_out=statsb[:, sa:sa + 1],
                        )
                        n_sa[ci] += 1
                        prev_ages[ci] = ages
                        prev_half[ci] = half
                    t0s[ci] += csize
            # per-chain a_end (chain-local age at the chain's last step)
            for ci in range(2):
                nc.vector.tensor_copy(
                    statsb[:, AEND0 + ci:AEND0 + ci + 1],
                    prev_ages[ci][:, prev_half[ci] - 1:prev_half[ci]],
                )
            nc.sync.dma_start(stats.ap(), statsb[:])

    nc.compile()
    return nc


_CACHE = {}


def _get_nc():
    if "nc" not in _CACHE:
        _CACHE["nc"] = build_kernel()
    return _CACHE["nc"]


_POP = np.array([bin(i).count("1") for i in range(16)], dtype=np.int64)


def _decode_bitmasks(bm, tt=TT):
    """bm: [128, >=tt/2] f16 of notm-nibbles -> per-time-half (k, t_f).

    Per chunk of size csize at t0, a [128, csize/2] block at free offset
    t0/2: partition p = 64*r + c (c < 32 real window), free = q*128 + n,
    blk = 2*q + r, covering t = t0 + 128*blk + 4*c + j with value
    sum(notm * 2^j); spike nibble is 15 - value.
    """
    parts = []
    off = 0
    bmv = np.asarray(bm, dtype=np.float64)
    sched = _chain_schedule(tt) * 2      # time order: chain A then chain B
    for csize in sched:
        bw = csize // 2
        qtot = csize // 256
        v = np.round(bmv[:, off:off + bw]).astype(np.int64)
        v = v.reshape(2, 64, qtot, 128)          # [r, c, q, n]
        m_nib = (15 - v[:, :32]).transpose(3, 2, 0, 1)  # [n, q, r, c]
        parts.append(m_nib.reshape(128, qtot * 2 * 32))
        off += bw
    flat = np.concatenate(parts, axis=1)         # [n, tt/4] time-ordered

    def half_stats(nib, base):
        k = _POP[nib].sum(axis=1)
        any_nib = nib > 0
        first_nib = np.argmax(any_nib, axis=1)
        has = any_nib.any(axis=1)
        nib_val = nib[np.arange(128), first_nib]
        low = np.zeros(128, dtype=np.int64)
        for j in range(3, -1, -1):
            low = np.where((nib_val >> j) & 1 == 1, j, low)
        t_f = np.where(has, base + first_nib * 4 + low, tt)
        return k.astype(np.float64), t_f.astype(np.float64)

    hn = flat.shape[1] // 2
    kA, tfA = half_stats(flat[:, :hn], 0)
    kB, tfB = half_stats(flat[:, hn:], tt // 2)
    return kA, tfA, kB, tfB


def _finalize(stats_list, bmask_list, target_cv, tt=TT):
    """Stitch the two time-half chains and compute the loss (host, f32)."""
    f32 = np.float32
    half = tt / 2.0
    k_l, tf_l, tl_l, s2_l = [], [], [], []
    nchain = len(_chain_schedule(tt))
    for st, bm in zip(stats_list, bmask_list):
        st = np.asarray(st, dtype=np.float64)
        sum_aA = st[:, SA0:SA0 + 2 * nchain].sum(axis=1)
        sum_aB = st[:, SA_B:SA_B + 2 * nchain].sum(axis=1)
        a_endA = st[:, AEND0]
        a_endB = st[:, AEND0 + 1]
        kA, tfA, kB, tfB = _decode_bitmasks(bm, tt)
        hasA, hasB = kA > 0, kB > 0
        # per-chain blind sums (chain-local time, init age 0)
        s2A = 1.0 - (a_endA + 1.0) ** 2 + 2.0 * sum_aA + half
        s2B = 1.0 - (a_endB + 1.0) ** 2 + 2.0 * sum_aB + half
        # remove each chain's blind first gap ((local t_f)+1)^2
        s2A = np.where(hasA, s2A - (tfA + 1.0) ** 2, 0.0)
        s2B = np.where(hasB, s2B - (tfB - half + 1.0) ** 2, 0.0)
        tlA = half - 1.0 - a_endA        # global (== local here)
        tlB = tt - 1.0 - a_endB
        # boundary gap between the halves
        s2 = s2A + s2B + np.where(
            hasA & hasB, (tfB - tlA) ** 2, 0.0
        )
        k = kA + kB
        t_f = np.where(hasA, tfA, tfB)   # == tt when no spikes at all
        t_l = np.where(hasB, tlB, tlA)   # == -1 when no spikes at all
        k_l.append(k); tf_l.append(t_f); tl_l.append(t_l); s2_l.append(s2)
    k = np.concatenate(k_l).astype(f32)
    t_f = np.concatenate(tf_l)
    t_l = np.concatenate(tl_l)
    s2 = np.concatenate(s2_l).astype(f32)
    tgt = np.asarray(target_cv, dtype=f32)

    n_isi = k - f32(1.0)
    sum_g = (t_l - t_f).astype(f32)
    mean = sum_g / np.maximum(n_isi, f32(1.0))
    var = (s2 - n_isi * mean * mean) / np.maximum(n_isi - f32(1.0), f32(1.0))
    std = np.sqrt(np.maximum(var, f32(0.0)).astype(f32))
    valid = (k >= f32(3.0)) & (mean > f32(0.0))
    cv = np.where(valid, std / np.where(mean > f32(0.0), mean, f32(1.0)), f32(0.0))
    sq = np.where(valid, (cv - tgt) ** 2, f32(0.0)).astype(f32)
    nvalid = valid.astype(f32).sum(dtype=f32)
    loss = np.where(
        nvalid > f32(0.0), sq.sum(dtype=f32) / np.maximum(nvalid, f32(1.0)), f32(0.0)
    )
    return np.asarray(loss, dtype=np.float32)


_IDENT = np.eye(128, dtype=np.float16)
_WMASK = _wmask_np()


def make_in_maps(output_spikes):
    s = np.asarray(output_spikes, dtype=np.float32).reshape(TT, N)
    return [
        {
            "spikes": np.ascontiguousarray(s[:, d * NLOC:(d + 1) * NLOC]),
            "ident": _IDENT,
            "wmask": _WMASK,
        }
        for d in range(NCORES)
    ]


def kernel(output_spikes, target_cv, _trace=False):
    nc = _get_nc()
    in_maps = make_in_maps(output_spikes)
    res = bass_utils.run_bass_kernel_spmd(
        nc, in_maps, core_ids=list(range(NCORES)), trace=_trace
    )
    _CACHE["last_result"] = res
    stats_list = [res.results[d]["stats"] for d in range(NCORES)]
    bmask_list = [res.results[d]["bmask"] for d in range(NCORES)]
    return _finalize(stats_list, bmask_list, target_cv)
